# revision 9
# baseline (speedup 1.0000x reference)
"""Trainium2 Bass kernel for nn_DataEmbedder (embedding_lookup).

Forward pass of a tabular data embedder:
  - dataset [64, 4096, 12] f32: cols 0-3 are raw categorical ids (stored as
    floats), cols 4-11 are numeric features.
  - For each categorical col k: ids -> lut_k remap -> emb_k gather.
  - Output [64, 4096, 128] = concat(emb0[32], emb1[64], emb2[16], emb3[8],
    numeric[8]).

Strategy (data-parallel over batch: 8 cores x 8 batch rows):
  Per-token gathers use the GPSIMD `dma_gather` custom instruction with
  256-byte elements and int16 indices wrapped in 16 partitions. The real
  bottleneck is GPSIMD descriptor generation, which serializes per gather
  instruction (~1us fixed cost each + per-descriptor time), so this version
  minimizes gather-instruction count:

  - All 4 embedding tables are stacked into ONE padded DRAM table
    (pemb_all/cemb_all [8192, 64] f32, 256B rows) with per-table row bases
    BASE=[0,1024,6144,6400]. Token ids get +BASE[k] folded into the
    f32->int16 conversion (tensor_scalar_add), so ONE dma_gather per chunk
    covers all 4 tables (idx slot layout [chunk][table][token]).
  - Stage A composes cemb_all = pemb_all[lut] with a single 6528-index
    gather (luts for all tables concatenated, values offset by BASE[k]).
  - Stage B: 8 chunks x 4096 tokens; per chunk one 16384-index gather into
    staging [128, 128, 64] (token t of table k lands at [t%128, k*32+t//128]),
    then assembly copies split across Vector and Scalar engines into the
    output tile [128, 32, 128], one 512B-row store DMA per chunk.
"""

import numpy as np

B, T = 64, 4096
NCORES = 8
BC = B // NCORES            # batch rows per core
N = BC * T                  # 32768 tokens per core
NCOLS = 12
VOCABS = [1000, 5000, 200, 50]
DIMS = [32, 64, 16, 8]
OFF = [0, 32, 96, 112]      # output column offset of each embedding block
NUM_OFF = 120               # numeric features start col
DOUT = 128
PAD = 64                    # padded row length (f32) = 256B
VPAD = [((v + 127) // 128) * 128 for v in VOCABS]   # 1024, 5120, 256, 128
BASE = [0, 1024, 6144, 6400]                        # row base in stacked table
CTOT = sum(VPAD)                                    # 6528 = 51*128
PROWS = 8192                                        # stacked table rows
LUT_SLOT_OFF = [0, 64, 384, 400]                    # int16 slots in lut region
LUT_REGION = 512                                    # lut slots (408 pad 512)

NCHUNK = 16
CH = N // NCHUNK            # 2048 tokens per chunk (4*CH = 8192 idx/gather,
                            # the max num_idxs one dma_gather survives on HW)
IPP = CH // 128             # 32 out slots per partition per chunk
SPT = CH // 16              # 256 idx slots per table per chunk
SPC = 4 * SPT               # 1024 idx slots per chunk
TOK_SLOTS = NCHUNK * SPC    # 8192
W16 = TOK_SLOTS + LUT_REGION

_CACHE = {}

SCRATCH = 65536     # SWDGE descriptor-ring capacity (descs)
GBUFS = 2
OBUFS = 2
NQUEUES = 1         # SWDGE queues (ucode max 4); gathers round-robin on them
SP = False          # dma_gather single_packet flag


def _build_program(reps=1):
    from contextlib import ExitStack

    import concourse.bacc as bacc
    import concourse.tile as tile
    from concourse import mybir
    from concourse.tile import add_dep_helper

    F32, I32, I16 = mybir.dt.float32, mybir.dt.int32, mybir.dt.int16

    nc = bacc.Bacc("TRN2", target_bir_lowering=False, debug=False,
                   num_devices=NCORES, dynamic_dma_scratch_size=SCRATCH,
                   num_swdge_queues=NQUEUES)
    ds = nc.dram_tensor("ds", [N, NCOLS], F32, kind="ExternalInput")
    out = nc.dram_tensor("out", [N, DOUT], F32, kind="ExternalOutput")
    embs = [
        nc.dram_tensor(f"emb{k}", [VOCABS[k], DIMS[k]], F32, kind="ExternalInput")
        for k in range(4)
    ]
    luts = [
        nc.dram_tensor(f"lut{k}", [VOCABS[k]], I32, kind="ExternalInput")
        for k in range(4)
    ]
    pemb = nc.dram_tensor("pemb_all", [PROWS, PAD], F32)
    cemb = nc.dram_tensor("cemb_all", [PROWS, PAD], F32)

    with tile.TileContext(nc) as tc:
        with ExitStack() as ctx:
            sm_pool = ctx.enter_context(tc.tile_pool(name="small", bufs=1))
            w_pool = ctx.enter_context(tc.tile_pool(name="widx", bufs=2))
            comp_pool = ctx.enter_context(tc.tile_pool(name="comp", bufs=1))
            nds_pool = ctx.enter_context(tc.tile_pool(name="nds", bufs=1))
            g_pool = ctx.enter_context(tc.tile_pool(name="gt", bufs=GBUFS))
            o_pool = ctx.enter_context(tc.tile_pool(name="ot", bufs=OBUFS))

            def one_pass():
                # ---------- Stage A: stacked padded table + lut composition ----
                pemb_cp = []
                for k in range(4):
                    w = nc.sync.dma_start(
                        out=pemb.ap()[BASE[k] : BASE[k] + VOCABS[k], : DIMS[k]],
                        in_=embs[k].ap(),
                    )
                    pemb_cp.append(w)

                # wrapped int16 index tile: [128, 8192 token | 512 lut slots]
                big16 = sm_pool.tile([128, W16], I16, name="big16")
                nc.vector.memset(big16[:16, TOK_SLOTS:], 0)

                # luts: load int32 wrapped (values pre-offset by BASE[k] on
                # the host), narrow to int16
                for k in range(4):
                    nslot = VPAD[k] // 16
                    wlut32 = sm_pool.tile([16, nslot], I32, name=f"wlut32_{k}")
                    nc.vector.memset(wlut32[:], 0)
                    V = VOCABS[k]
                    m, tail = V // 16, V % 16
                    nc.sync.dma_start(
                        out=wlut32[:, :m],
                        in_=luts[k].ap()[: 16 * m].rearrange("(s r) -> r s", r=16),
                    )
                    if tail:
                        nc.sync.dma_start(
                            out=wlut32[:tail, m : m + 1],
                            in_=luts[k].ap()[16 * m :].rearrange("(s r) -> r s", r=tail),
                        )
                    lo = TOK_SLOTS + LUT_SLOT_OFF[k]
                    nc.vector.tensor_copy(
                        out=big16[:16, lo : lo + nslot], in_=wlut32[:]
                    )

                # token id cols: wrapped load per chunk (cat cols pre-offset
                # by BASE[k] on the host), f32->int16 conversion
                for c in range(NCHUNK):
                    widx = w_pool.tile([16, SPT, 4], F32, name="widx")
                    nc.sync.dma_start(
                        out=widx[:],
                        in_=ds.ap()[c * CH : (c + 1) * CH, 0:4].rearrange(
                            "(s r) k -> r s k", r=16
                        ),
                    )
                    for k in range(4):
                        so = c * SPC + k * SPT
                        nc.vector.tensor_copy(
                            out=big16[:16, so : so + SPT],
                            in_=widx[:, :, k],
                        )

                # replicate partitions 0:16 -> 0:128 by doubling
                nc.sync.dma_start(out=big16[16:32, :], in_=big16[0:16, :])
                nc.sync.dma_start(out=big16[32:64, :], in_=big16[0:32, :])
                nc.sync.dma_start(out=big16[64:128, :], in_=big16[0:64, :])

                # lut composition: cemb = pemb[lut_all] in one gather
                comp_t = comp_pool.tile([128, CTOT // 128, PAD], F32, name="comp_t")
                cg = nc.gpsimd.dma_gather(
                    comp_t[:],
                    pemb.ap(),
                    big16[:, TOK_SLOTS : TOK_SLOTS + CTOT // 16],
                    CTOT,
                    CTOT,
                    PAD,
                    single_packet=SP,
                )
                for w in pemb_cp:
                    add_dep_helper(cg.ins, w.ins, reason="pemb RAW")
                wb = nc.sync.dma_start(
                    out=cemb.ap()[:CTOT, :].rearrange("(i p) d -> p i d", p=128),
                    in_=comp_t[:],
                )

                # numeric features, already in the [p=n%128, i=n//128] layout
                nds = nds_pool.tile([128, N // 128, 8], F32, name="nds")
                nc.sync.dma_start(
                    out=nds[:],
                    in_=ds.ap()[:, 4:NCOLS].rearrange("(i p) k -> p i k", p=128),
                )

                # ---------- Stage B: per-chunk merged gather + assemble + store
                for c in range(NCHUNK):
                    o_t = o_pool.tile([128, IPP, DOUT], F32, name="o_t")
                    nc.scalar.copy(
                        out=o_t[:, :, NUM_OFF:],
                        in_=nds[:, c * IPP : (c + 1) * IPP, :],
                    )
                    g_t = g_pool.tile([128, 4 * IPP, PAD], F32, name="g_t")
                    gi = nc.gpsimd.dma_gather(
                        g_t[:],
                        cemb.ap(),
                        big16[:, c * SPC : (c + 1) * SPC],
                        4 * CH,
                        4 * CH,
                        PAD,
                        single_packet=SP,
                        queue_num=c % NQUEUES,
                    )
                    add_dep_helper(gi.ins, wb.ins, reason="cemb RAW")
                    for k in range(4):
                        src = g_t[:, k * IPP : (k + 1) * IPP, : DIMS[k]]
                        dst = o_t[:, :, OFF[k] : OFF[k] + DIMS[k]]
                        if k == 1:
                            nc.scalar.copy(out=dst, in_=src)
                        else:
                            nc.vector.tensor_copy(out=dst, in_=src)
                    nc.sync.dma_start(
                        out=out.ap()[c * CH : (c + 1) * CH, :].rearrange(
                            "(i p) f -> p i f", p=128
                        ),
                        in_=o_t[:],
                    )

            for _rep in range(reps):
                one_pass()
    nc.compile()
    return nc


def get_program():
    if "nc" not in _CACHE:
        _CACHE["nc"] = _build_program()
    return _CACHE["nc"]


def make_in_maps(inputs):
    # Fold the stacked-table row bases into the categorical ids / lut values
    # during input marshalling so the on-chip int16 index build is a plain
    # dtype-converting copy.
    dataset = np.array(np.asarray(inputs["dataset"], dtype=np.float32))
    dataset[:, :, 0:4] += np.asarray(BASE, dtype=np.float32)
    in_maps = []
    for i in range(NCORES):
        m = {
            "ds": np.ascontiguousarray(
                dataset[i * BC : (i + 1) * BC].reshape(N, NCOLS)
            )
        }
        for k in range(4):
            m[f"emb{k}"] = np.ascontiguousarray(inputs[f"emb{k}"], dtype=np.float32)
            m[f"lut{k}"] = np.ascontiguousarray(
                np.asarray(inputs[f"lut{k}"], dtype=np.int32) + BASE[k]
            )
        in_maps.append(m)
    return in_maps


def kernel(**inputs):
    from concourse.bass_utils import run_bass_kernel_spmd

    nc = get_program()
    in_maps = make_in_maps(inputs)
    res = run_bass_kernel_spmd(nc, in_maps, list(range(NCORES))).results
    outs = [np.asarray(res[i]["out"]).reshape(BC, T, DOUT) for i in range(NCORES)]
    return np.concatenate(outs, axis=0)


# revision 11
# speedup vs baseline: 1.1546x; 1.1546x over previous
"""Trainium2 Bass kernel for nn_DataEmbedder (embedding_lookup).

Forward pass of a tabular data embedder:
  - dataset [64, 4096, 12] f32: cols 0-3 are raw categorical ids (stored as
    floats), cols 4-11 are numeric features.
  - For each categorical col k: ids -> lut_k remap -> emb_k gather.
  - Output [64, 4096, 128] = concat(emb0[32], emb1[64], emb2[16], emb3[8],
    numeric[8]).

Strategy (data-parallel over batch: 8 cores x 8 batch rows):
  The wall is GPSIMD SWDGE descriptor generation (~8ns per gather
  descriptor, serial on the Q7 complex), so per-token dma_gather is used
  only for the two big tables (emb0, emb1); the small tables emb2 (200x16)
  and emb3 (50x8) are gathered with TensorE one-hot matmuls instead:

  - one-hot[v, tok] = (ids_bcast == iota_partition) computed on DVE in bf16
    (ids row broadcast to 128 partitions via one stride-0 SBUF->SBUF DMA),
  - psum[tok, d] = onehot.T @ emb_bf16 -- exact row select (single nonzero
    per column; bf16 rounding of table values is ~0.4% rel err, well under
    the 2e-2 gate). emb2 contracts over two partition chunks (128+72) with
    PSUM accumulation.

  Stage A composes lut-applied padded tables (cemb_k = pemb_k[lut_k]) via
  one small dma_gather per table; only emb0/emb1 are written back to DRAM
  for token gathers, emb2/emb3 feed the PE path straight from SBUF.
"""

import numpy as np

B, T = 64, 4096
NCORES = 8
BC = B // NCORES            # batch rows per core
N = BC * T                  # 32768 tokens per core
NCOLS = 12
VOCABS = [1000, 5000, 200, 50]
DIMS = [32, 64, 16, 8]
OFF = [0, 32, 96, 112]      # output column offset of each embedding block
NUM_OFF = 120               # numeric features start col
DOUT = 128
PAD = 64                    # padded row length (f32) = 256B
PROWS = 8192
VPAD = [((v + 127) // 128) * 128 for v in VOCABS]   # 1024, 5120, 256, 128
LUT_SLOT_OFF = [0, 64, 384, 400]                    # int16 slots in lut region
LUT_REGION = 512

NCHUNK = 16
CH = N // NCHUNK            # 2048 tokens per chunk
IPP = CH // 128             # 16 out slots per partition per chunk
SPC = CH // 16              # 128 wrapped idx slots per table per chunk
TOK_SLOTS = 2 * (N // 16)   # t0 + t1 idx regions
W16 = TOK_SLOTS + LUT_REGION

_CACHE = {}

SCRATCH = 65536
GBUFS = 4
OBUFS = 2
NQUEUES = 1
BCAST_DMA = False   # stride-0 partition DMA unsupported; use doubling DMAs


def _build_program(reps=1):
    from contextlib import ExitStack

    import concourse.bacc as bacc
    import concourse.tile as tile
    from concourse import mybir
    from concourse.tile import add_dep_helper

    F32, I32, I16 = mybir.dt.float32, mybir.dt.int32, mybir.dt.int16
    BF16 = mybir.dt.bfloat16

    nc = bacc.Bacc("TRN2", target_bir_lowering=False, debug=False,
                   num_devices=NCORES, dynamic_dma_scratch_size=SCRATCH,
                   num_swdge_queues=NQUEUES)
    ds = nc.dram_tensor("ds", [N, NCOLS], F32, kind="ExternalInput")
    out = nc.dram_tensor("out", [N, DOUT], F32, kind="ExternalOutput")
    embs = [
        nc.dram_tensor(f"emb{k}", [VOCABS[k], DIMS[k]], F32, kind="ExternalInput")
        for k in range(4)
    ]
    luts = [
        nc.dram_tensor(f"lut{k}", [VOCABS[k]], I32, kind="ExternalInput")
        for k in range(4)
    ]
    pembs = [nc.dram_tensor(f"pemb{k}", [PROWS, PAD], F32) for k in range(4)]
    cembs = [nc.dram_tensor(f"cemb{k}", [PROWS, PAD], F32) for k in range(2)]

    with tile.TileContext(nc) as tc:
        with ExitStack() as ctx:
            sm_pool = ctx.enter_context(tc.tile_pool(name="small", bufs=1))
            comp_pool = ctx.enter_context(tc.tile_pool(name="comp", bufs=1))
            nds_pool = ctx.enter_context(tc.tile_pool(name="nds", bufs=1))
            g_pool = ctx.enter_context(tc.tile_pool(name="gt", bufs=GBUFS))
            o_pool = ctx.enter_context(tc.tile_pool(name="ot", bufs=OBUFS))
            idr_pool = ctx.enter_context(tc.tile_pool(name="idr", bufs=2))
            rep_pool = ctx.enter_context(tc.tile_pool(name="idsrep", bufs=2))
            oh_pool = ctx.enter_context(tc.tile_pool(name="oh", bufs=2))
            ps_pool = ctx.enter_context(
                tc.tile_pool(name="ps", bufs=2, space="PSUM"))

            def one_pass():
                # ---------- Stage A ----------
                pemb_cp = []
                for k in range(4):
                    w = nc.sync.dma_start(
                        out=pembs[k].ap()[: VOCABS[k], : DIMS[k]],
                        in_=embs[k].ap(),
                    )
                    pemb_cp.append(w)

                big16 = sm_pool.tile([128, W16], I16, name="big16")
                nc.vector.memset(big16[:16, TOK_SLOTS:], 0)

                for k in range(4):
                    nslot = VPAD[k] // 16
                    wlut32 = sm_pool.tile([16, nslot], I32, name=f"wlut32_{k}")
                    nc.vector.memset(wlut32[:], 0)
                    V = VOCABS[k]
                    m, tail = V // 16, V % 16
                    nc.sync.dma_start(
                        out=wlut32[:, :m],
                        in_=luts[k].ap()[: 16 * m].rearrange("(s r) -> r s", r=16),
                    )
                    if tail:
                        nc.sync.dma_start(
                            out=wlut32[:tail, m : m + 1],
                            in_=luts[k].ap()[16 * m :].rearrange("(s r) -> r s", r=tail),
                        )
                    lo = TOK_SLOTS + LUT_SLOT_OFF[k]
                    nc.vector.tensor_copy(
                        out=big16[:16, lo : lo + nslot], in_=wlut32[:]
                    )

                # token id cols 0,1 wrapped
                widx = sm_pool.tile([16, N // 16, 2], F32, name="widx")
                nc.sync.dma_start(
                    out=widx[:],
                    in_=ds.ap()[:, 0:2].rearrange("(s r) k -> r s k", r=16),
                )
                for k in range(2):
                    nc.vector.tensor_copy(
                        out=big16[:16, k * (N // 16) : (k + 1) * (N // 16)],
                        in_=widx[:, :, k],
                    )

                nc.sync.dma_start(out=big16[16:32, :], in_=big16[0:16, :])
                nc.sync.dma_start(out=big16[32:64, :], in_=big16[0:32, :])
                nc.sync.dma_start(out=big16[64:128, :], in_=big16[0:64, :])

                # compose cemb_k = pemb_k[lut_k]
                comp_ts = []
                wb = []
                for k in range(4):
                    nslot = VPAD[k] // 16
                    lo = TOK_SLOTS + LUT_SLOT_OFF[k]
                    comp_t = comp_pool.tile(
                        [128, VPAD[k] // 128, PAD], F32, name=f"comp_t{k}"
                    )
                    cg = nc.gpsimd.dma_gather(
                        comp_t[:],
                        pembs[k].ap(),
                        big16[:, lo : lo + nslot],
                        VPAD[k],
                        VPAD[k],
                        PAD,
                        single_packet=False,
                    )
                    add_dep_helper(cg.ins, pemb_cp[k].ins, reason=f"pemb{k} RAW")
                    comp_ts.append(comp_t)
                    if k < 2:
                        w = nc.sync.dma_start(
                            out=cembs[k].ap()[: VPAD[k], :].rearrange(
                                "(i p) d -> p i d", p=128
                            ),
                            in_=comp_t[:],
                        )
                        wb.append(w)

                # bf16 moving operands for the PE path (vocab chunks are
                # 128-row aligned in comp tiles)
                mv2a = sm_pool.tile([128, 16], BF16, name="mv2a")
                nc.vector.tensor_copy(out=mv2a[:], in_=comp_ts[2][:, 0, :16])
                mv2b = sm_pool.tile([128, 16], BF16, name="mv2b")
                nc.vector.tensor_copy(out=mv2b[:72, :], in_=comp_ts[2][0:72, 1, :16])
                mv3 = sm_pool.tile([128, 8], BF16, name="mv3")
                nc.vector.tensor_copy(out=mv3[:50, :], in_=comp_ts[3][0:50, 0, :8])

                # per-partition iota columns for the one-hot compares
                io0 = sm_pool.tile([128, 1], F32, name="io0")
                nc.gpsimd.iota(io0[:], pattern=[[0, 1]], base=0,
                               channel_multiplier=1,
                               allow_small_or_imprecise_dtypes=True)
                io1 = sm_pool.tile([128, 1], F32, name="io1")
                nc.gpsimd.iota(io1[:], pattern=[[0, 1]], base=128,
                               channel_multiplier=1,
                               allow_small_or_imprecise_dtypes=True)

                # numeric features
                nds = nds_pool.tile([128, N // 128, 8], F32, name="nds")
                nc.sync.dma_start(
                    out=nds[:],
                    in_=ds.ap()[:, 4:NCOLS].rearrange("(i p) k -> p i k", p=128),
                )

                # ---------- Stage B ----------
                for c in range(NCHUNK):
                    o_t = o_pool.tile([128, IPP, DOUT], F32, name="o_t")
                    nc.scalar.copy(
                        out=o_t[:, :, NUM_OFF:],
                        in_=nds[:, c * IPP : (c + 1) * IPP, :],
                    )
                    # big-table token gathers
                    for k in range(2):
                        g_t = g_pool.tile([128, IPP, PAD], F32, name="g_t")
                        so = k * (N // 16) + c * SPC
                        gi = nc.gpsimd.dma_gather(
                            g_t[:],
                            cembs[k].ap(),
                            big16[:, so : so + SPC],
                            CH,
                            CH,
                            PAD,
                            single_packet=False,
                        )
                        add_dep_helper(gi.ins, wb[k].ins, reason=f"cemb{k} RAW")
                        if k == 0:
                            nc.vector.tensor_copy(
                                out=o_t[:, :, OFF[0] : OFF[0] + DIMS[0]],
                                in_=g_t[:, :, : DIMS[0]],
                            )
                        else:
                            nc.scalar.copy(
                                out=o_t[:, :, OFF[1] : OFF[1] + DIMS[1]],
                                in_=g_t[:, :, : DIMS[1]],
                            )

                    # small tables via one-hot matmul
                    idr = idr_pool.tile([2, CH], F32, name="idr")
                    nc.sync.dma_start(
                        out=idr[:],
                        in_=ds.ap()[c * CH : (c + 1) * CH, 2:4].rearrange(
                            "n k -> k n"
                        ),
                    )
                    rep2 = rep_pool.tile([128, CH], F32, name="rep2")
                    rep3 = rep_pool.tile([128, CH], F32, name="rep3")
                    if BCAST_DMA:
                        nc.sync.dma_start(
                            out=rep2[:],
                            in_=idr[0:1, :].broadcast_to([128, CH]),
                        )
                        nc.sync.dma_start(
                            out=rep3[:],
                            in_=idr[1:2, :].broadcast_to([128, CH]),
                        )
                    else:
                        nc.sync.dma_start(out=rep2[0:1, :], in_=idr[0:1, :])
                        nc.sync.dma_start(out=rep3[0:1, :], in_=idr[1:2, :])
                        for m in (1, 2, 4, 8, 16, 32, 64):
                            nc.sync.dma_start(
                                out=rep2[m : 2 * m, :], in_=rep2[0:m, :])
                            nc.scalar.dma_start(
                                out=rep3[m : 2 * m, :], in_=rep3[0:m, :])

                    oh2a = oh_pool.tile([128, CH], BF16, name="oh2a")
                    nc.vector.tensor_scalar(
                        out=oh2a[:], in0=rep2[:], scalar1=io0[:], scalar2=None,
                        op0=mybir.AluOpType.is_equal,
                    )
                    oh2b = oh_pool.tile([128, CH], BF16, name="oh2b")
                    nc.vector.tensor_scalar(
                        out=oh2b[0:72, :], in0=rep2[0:72, :], scalar1=io1[0:72, :],
                        scalar2=None, op0=mybir.AluOpType.is_equal,
                    )
                    oh3 = oh_pool.tile([128, CH], BF16, name="oh3")
                    nc.vector.tensor_scalar(
                        out=oh3[0:50, :], in0=rep3[0:50, :], scalar1=io0[0:50, :],
                        scalar2=None, op0=mybir.AluOpType.is_equal,
                    )

                    pp = ps_pool.tile([128, IPP, 24], F32, name="pp")
                    for ti in range(IPP):
                        ts = ti * 128
                        nc.tensor.matmul(
                            pp[:, ti, 0:16],
                            oh2a[:, ts : ts + 128],
                            mv2a[:],
                            start=True, stop=False, skip_group_check=True,
                        )
                        nc.tensor.matmul(
                            pp[:, ti, 0:16],
                            oh2b[0:72, ts : ts + 128],
                            mv2b[0:72, :],
                            start=False, stop=True, skip_group_check=True,
                        )
                        nc.tensor.matmul(
                            pp[:, ti, 16:24],
                            oh3[0:50, ts : ts + 128],
                            mv3[0:50, :],
                            start=True, stop=True, skip_group_check=True,
                        )
                    nc.scalar.copy(
                        out=o_t[:, :, OFF[2] : OFF[2] + 24], in_=pp[:]
                    )

                    nc.sync.dma_start(
                        out=out.ap()[c * CH : (c + 1) * CH, :].rearrange(
                            "(i p) f -> p i f", p=128
                        ),
                        in_=o_t[:],
                    )

            for _rep in range(reps):
                one_pass()
    nc.compile()
    return nc


def get_program():
    if "nc" not in _CACHE:
        _CACHE["nc"] = _build_program()
    return _CACHE["nc"]


def make_in_maps(inputs):
    dataset = np.asarray(inputs["dataset"], dtype=np.float32)
    in_maps = []
    for i in range(NCORES):
        m = {
            "ds": np.ascontiguousarray(
                dataset[i * BC : (i + 1) * BC].reshape(N, NCOLS)
            )
        }
        for k in range(4):
            m[f"emb{k}"] = np.ascontiguousarray(inputs[f"emb{k}"], dtype=np.float32)
            m[f"lut{k}"] = np.ascontiguousarray(inputs[f"lut{k}"], dtype=np.int32)
        in_maps.append(m)
    return in_maps


def kernel(**inputs):
    from concourse.bass_utils import run_bass_kernel_spmd

    nc = get_program()
    in_maps = make_in_maps(inputs)
    res = run_bass_kernel_spmd(nc, in_maps, list(range(NCORES))).results
    outs = [np.asarray(res[i]["out"]).reshape(BC, T, DOUT) for i in range(NCORES)]
    return np.concatenate(outs, axis=0)


# revision 13
# speedup vs baseline: 2.0907x; 1.8108x over previous
"""Trainium2 Bass kernel for nn_DataEmbedder (embedding_lookup).

Forward pass of a tabular data embedder:
  - dataset [64, 4096, 12] f32: cols 0-3 raw categorical ids (as floats),
    cols 4-11 numeric features.
  - For each categorical col k: ids -> lut_k remap -> emb_k gather.
  - Output [64, 4096, 128] = concat(emb0[32], emb1[64], emb2[16], emb3[8],
    numeric[8]).

Strategy (data-parallel over batch: 8 cores x 8 batch rows). Two walls on
this part, both ~8-9ns per DMA descriptor: SWDGE (GPSIMD dma_gather) and
HWDGE (regular dma_start) descriptor generation. So:

  - Tables emb0/emb1 use per-token dma_gather (2048-idx per chunk, the
    per-descriptor sweet spot); emb2 (200x16) / emb3 (50x8) are gathered
    with TensorE one-hot matmuls (bf16, exact row-select, ~0.4% quant err
    vs the 2e-2 gate), removing half the SWDGE descriptors.
  - Every dma_start is made contiguous-per-partition so HWDGE descriptor
    counts collapse (~185k -> ~15k): the host marshals pre-wrapped index
    arrays (idsw0/idsw1), a j-ordered id row pair (ids23), a p-major
    numeric block (dsnum), and pre-wrapped luts; tokens are assigned to
    gather positions p-major (token = p*256 + c*16 + slot) so each output
    store is one 8KB contiguous run per partition (128 descs vs 2048).
"""

import numpy as np

B, T = 64, 4096
NCORES = 8
BC = B // NCORES            # batch rows per core
N = BC * T                  # 32768 tokens per core
NCOLS = 12
VOCABS = [1000, 5000, 200, 50]
DIMS = [32, 64, 16, 8]
OFF = [0, 32, 96, 112]
NUM_OFF = 120
DOUT = 128
PAD = 64                    # padded row length (f32) = 256B
PROWS = 8192
VPAD = [((v + 127) // 128) * 128 for v in VOCABS]   # 1024, 5120, 256, 128
LUT_SLOT_OFF = [0, 64, 384, 400]
LUT_REGION = 512

NCHUNK = 16
CH = N // NCHUNK            # 2048 tokens per chunk
IPP = CH // 128             # 16 out slots per partition per chunk
SPC = CH // 16              # 128 idx slots per table per chunk
TOK_SLOTS = 2 * (N // 16)
W16 = TOK_SLOTS + LUT_REGION

_CACHE = {}

SCRATCH = 65536
GBUFS = 4
OBUFS = 2
NQUEUES = 1


def _build_program(reps=1):
    from contextlib import ExitStack

    import concourse.bacc as bacc
    import concourse.tile as tile
    from concourse import mybir
    from concourse.tile import add_dep_helper

    F32, I32, I16 = mybir.dt.float32, mybir.dt.int32, mybir.dt.int16
    BF16 = mybir.dt.bfloat16

    nc = bacc.Bacc("TRN2", target_bir_lowering=False, debug=False,
                   num_devices=NCORES, dynamic_dma_scratch_size=SCRATCH,
                   num_swdge_queues=NQUEUES)
    idsw_d = [nc.dram_tensor(f"idsw{k}", [16, N // 16], I32, kind="ExternalInput")
              for k in range(2)]
    ids23_d = nc.dram_tensor("ids23", [2, N], F32, kind="ExternalInput")
    dsnum_d = nc.dram_tensor("dsnum", [128, N // 128, 8], F32,
                             kind="ExternalInput")
    lutw_d = [nc.dram_tensor(f"lutw{k}", [16, VPAD[k] // 16], I32,
                             kind="ExternalInput")
              for k in range(4)]
    out = nc.dram_tensor("out", [N, DOUT], F32, kind="ExternalOutput")
    embs = [
        nc.dram_tensor(f"emb{k}", [VOCABS[k], DIMS[k]], F32, kind="ExternalInput")
        for k in range(4)
    ]
    pembs = [nc.dram_tensor(f"pemb{k}", [PROWS, PAD], F32) for k in range(4)]
    cembs = [nc.dram_tensor(f"cemb{k}", [PROWS, PAD], F32) for k in range(2)]

    with tile.TileContext(nc) as tc:
        with ExitStack() as ctx:
            sm_pool = ctx.enter_context(tc.tile_pool(name="small", bufs=1))
            comp_pool = ctx.enter_context(tc.tile_pool(name="comp", bufs=1))
            nds_pool = ctx.enter_context(tc.tile_pool(name="nds", bufs=1))
            g_pool = ctx.enter_context(tc.tile_pool(name="gt", bufs=GBUFS))
            o_pool = ctx.enter_context(tc.tile_pool(name="ot", bufs=OBUFS))
            idr_pool = ctx.enter_context(tc.tile_pool(name="idr", bufs=2))
            rep_pool = ctx.enter_context(tc.tile_pool(name="idsrep", bufs=2))
            oh_pool = ctx.enter_context(tc.tile_pool(name="oh", bufs=2))
            ps_pool = ctx.enter_context(
                tc.tile_pool(name="ps", bufs=2, space="PSUM"))

            def one_pass():
                # ---------- Stage A ----------
                pemb_cp = []
                for k in range(4):
                    w = nc.sync.dma_start(
                        out=pembs[k].ap()[: VOCABS[k], : DIMS[k]],
                        in_=embs[k].ap(),
                    )
                    pemb_cp.append(w)

                big16 = sm_pool.tile([128, W16], I16, name="big16")
                nc.vector.memset(big16[:16, TOK_SLOTS + 408 :], 0)

                # luts: host-wrapped int32 -> int16
                for k in range(4):
                    nslot = VPAD[k] // 16
                    wlut32 = sm_pool.tile([16, nslot], I32, name=f"wlut32_{k}")
                    nc.sync.dma_start(out=wlut32[:], in_=lutw_d[k].ap())
                    lo = TOK_SLOTS + LUT_SLOT_OFF[k]
                    nc.vector.tensor_copy(
                        out=big16[:16, lo : lo + nslot], in_=wlut32[:]
                    )

                # token ids for t0/t1: host-wrapped int32 -> int16
                for k in range(2):
                    widx32 = sm_pool.tile([16, N // 16], I32, name=f"widx32_{k}")
                    nc.sync.dma_start(out=widx32[:], in_=idsw_d[k].ap())
                    nc.vector.tensor_copy(
                        out=big16[:16, k * (N // 16) : (k + 1) * (N // 16)],
                        in_=widx32[:],
                    )

                nc.sync.dma_start(out=big16[16:32, :], in_=big16[0:16, :])
                nc.sync.dma_start(out=big16[32:64, :], in_=big16[0:32, :])
                nc.sync.dma_start(out=big16[64:128, :], in_=big16[0:64, :])

                # compose cemb_k = pemb_k[lut_k]
                comp_ts = []
                wb = []
                for k in range(4):
                    nslot = VPAD[k] // 16
                    lo = TOK_SLOTS + LUT_SLOT_OFF[k]
                    comp_t = comp_pool.tile(
                        [128, VPAD[k] // 128, PAD], F32, name=f"comp_t{k}"
                    )
                    cg = nc.gpsimd.dma_gather(
                        comp_t[:],
                        pembs[k].ap(),
                        big16[:, lo : lo + nslot],
                        VPAD[k],
                        VPAD[k],
                        PAD,
                        single_packet=False,
                    )
                    add_dep_helper(cg.ins, pemb_cp[k].ins, reason=f"pemb{k} RAW")
                    comp_ts.append(comp_t)
                    if k < 2:
                        w = nc.sync.dma_start(
                            out=cembs[k].ap()[: VPAD[k], :].rearrange(
                                "(i p) d -> p i d", p=128
                            ),
                            in_=comp_t[:],
                        )
                        wb.append(w)

                # bf16 moving operands for the PE path
                mv2a = sm_pool.tile([128, 16], BF16, name="mv2a")
                nc.vector.tensor_copy(out=mv2a[:], in_=comp_ts[2][:, 0, :16])
                mv2b = sm_pool.tile([128, 16], BF16, name="mv2b")
                nc.vector.tensor_copy(out=mv2b[:72, :], in_=comp_ts[2][0:72, 1, :16])
                mv3 = sm_pool.tile([128, 8], BF16, name="mv3")
                nc.vector.tensor_copy(out=mv3[:50, :], in_=comp_ts[3][0:50, 0, :8])

                io0 = sm_pool.tile([128, 1], F32, name="io0")
                nc.gpsimd.iota(io0[:], pattern=[[0, 1]], base=0,
                               channel_multiplier=1,
                               allow_small_or_imprecise_dtypes=True)
                io1 = sm_pool.tile([128, 1], F32, name="io1")
                nc.gpsimd.iota(io1[:], pattern=[[0, 1]], base=128,
                               channel_multiplier=1,
                               allow_small_or_imprecise_dtypes=True)

                # numeric features (host p-major block, contiguous load)
                nds = nds_pool.tile([128, N // 128, 8], F32, name="nds")
                nc.sync.dma_start(out=nds[:], in_=dsnum_d.ap())

                out_pm = out.ap().rearrange("(p i) f -> p i f", p=128)

                # ---------- Stage B ----------
                for c in range(NCHUNK):
                    o_t = o_pool.tile([128, IPP, DOUT], F32, name="o_t")
                    nc.scalar.copy(
                        out=o_t[:, :, NUM_OFF:],
                        in_=nds[:, c * IPP : (c + 1) * IPP, :],
                    )
                    for k in range(2):
                        g_t = g_pool.tile([128, IPP, PAD], F32, name="g_t")
                        so = k * (N // 16) + c * SPC
                        gi = nc.gpsimd.dma_gather(
                            g_t[:],
                            cembs[k].ap(),
                            big16[:, so : so + SPC],
                            CH,
                            CH,
                            PAD,
                            single_packet=False,
                        )
                        add_dep_helper(gi.ins, wb[k].ins, reason=f"cemb{k} RAW")
                        if k == 0:
                            nc.vector.tensor_copy(
                                out=o_t[:, :, OFF[0] : OFF[0] + DIMS[0]],
                                in_=g_t[:, :, : DIMS[0]],
                            )
                        else:
                            nc.scalar.copy(
                                out=o_t[:, :, OFF[1] : OFF[1] + DIMS[1]],
                                in_=g_t[:, :, : DIMS[1]],
                            )

                    # small tables via one-hot matmul
                    idr = idr_pool.tile([2, CH], F32, name="idr")
                    nc.sync.dma_start(
                        out=idr[:], in_=ids23_d.ap()[:, c * CH : (c + 1) * CH]
                    )
                    rep2 = rep_pool.tile([128, CH], F32, name="rep2")
                    rep3 = rep_pool.tile([128, CH], F32, name="rep3")
                    nc.sync.dma_start(out=rep2[0:1, :], in_=idr[0:1, :])
                    nc.sync.dma_start(out=rep3[0:1, :], in_=idr[1:2, :])
                    for m in (1, 2, 4, 8, 16, 32, 64):
                        nc.sync.dma_start(out=rep2[m : 2 * m, :], in_=rep2[0:m, :])
                        nc.scalar.dma_start(out=rep3[m : 2 * m, :], in_=rep3[0:m, :])

                    oh2a = oh_pool.tile([128, CH], BF16, name="oh2a")
                    nc.vector.tensor_scalar(
                        out=oh2a[:], in0=rep2[:], scalar1=io0[:], scalar2=None,
                        op0=mybir.AluOpType.is_equal,
                    )
                    oh2b = oh_pool.tile([128, CH], BF16, name="oh2b")
                    nc.vector.tensor_scalar(
                        out=oh2b[0:72, :], in0=rep2[0:72, :], scalar1=io1[0:72, :],
                        scalar2=None, op0=mybir.AluOpType.is_equal,
                    )
                    oh3 = oh_pool.tile([128, CH], BF16, name="oh3")
                    nc.vector.tensor_scalar(
                        out=oh3[0:50, :], in0=rep3[0:50, :], scalar1=io0[0:50, :],
                        scalar2=None, op0=mybir.AluOpType.is_equal,
                    )

                    pp = ps_pool.tile([128, IPP, 24], F32, name="pp")
                    for ti in range(IPP):
                        ts = ti * 128
                        nc.tensor.matmul(
                            pp[:, ti, 0:16],
                            oh2a[:, ts : ts + 128],
                            mv2a[:],
                            start=True, stop=False, skip_group_check=True,
                        )
                        nc.tensor.matmul(
                            pp[:, ti, 0:16],
                            oh2b[0:72, ts : ts + 128],
                            mv2b[0:72, :],
                            start=False, stop=True, skip_group_check=True,
                        )
                        nc.tensor.matmul(
                            pp[:, ti, 16:24],
                            oh3[0:50, ts : ts + 128],
                            mv3[0:50, :],
                            start=True, stop=True, skip_group_check=True,
                        )
                    nc.scalar.copy(
                        out=o_t[:, :, OFF[2] : OFF[2] + 24], in_=pp[:]
                    )

                    nc.sync.dma_start(
                        out=out_pm[:, c * IPP : (c + 1) * IPP, :],
                        in_=o_t[:],
                    )

            for _rep in range(reps):
                one_pass()
    nc.compile()
    return nc


def get_program():
    if "nc" not in _CACHE:
        _CACHE["nc"] = _build_program()
    return _CACHE["nc"]


# token assigned to gather position: chunk c, in-chunk position j ->
# token (j%128)*256 + c*16 + j//128; global position g = c*2048 + j.
def _token_perm():
    g = np.arange(N)
    c, j = g // CH, g % CH
    return (j % 128) * 256 + c * IPP + j // 128


_TPERM = _token_perm()


def make_in_maps(inputs):
    dataset = np.asarray(inputs["dataset"], dtype=np.float32)
    in_maps = []
    for i in range(NCORES):
        dsc = dataset[i * BC : (i + 1) * BC].reshape(N, NCOLS)
        m = {}
        ids01 = dsc[:, 0:2].astype(np.int32)
        for k in range(2):
            x = ids01[_TPERM, k]
            m[f"idsw{k}"] = np.ascontiguousarray(x.reshape(N // 16, 16).T)
        m["ids23"] = np.ascontiguousarray(dsc[_TPERM, 2:4].T)
        m["dsnum"] = np.ascontiguousarray(dsc[:, 4:NCOLS].reshape(128, N // 128, 8))
        for k in range(4):
            lut = np.zeros(VPAD[k], dtype=np.int32)
            lut[: VOCABS[k]] = np.asarray(inputs[f"lut{k}"], dtype=np.int32)
            m[f"lutw{k}"] = np.ascontiguousarray(lut.reshape(VPAD[k] // 16, 16).T)
            m[f"emb{k}"] = np.ascontiguousarray(inputs[f"emb{k}"], dtype=np.float32)
        in_maps.append(m)
    return in_maps


def kernel(**inputs):
    from concourse.bass_utils import run_bass_kernel_spmd

    nc = get_program()
    in_maps = make_in_maps(inputs)
    res = run_bass_kernel_spmd(nc, in_maps, list(range(NCORES))).results
    outs = [np.asarray(res[i]["out"]).reshape(BC, T, DOUT) for i in range(NCORES)]
    return np.concatenate(outs, axis=0)


# revision 16
# speedup vs baseline: 2.4222x; 1.1585x over previous
"""Trainium2 Bass kernel for nn_DataEmbedder (embedding_lookup).

Forward pass of a tabular data embedder:
  - dataset [64, 4096, 12] f32: cols 0-3 raw categorical ids (as floats),
    cols 4-11 numeric features.
  - For each categorical col k: ids -> lut_k remap -> emb_k gather.
  - Output [64, 4096, 128] = concat(emb0[32], emb1[64], emb2[16], emb3[8],
    numeric[8]).

Strategy (data-parallel over batch: 8 cores x 8 batch rows). Two walls on
this part, both ~8-9ns per DMA descriptor: SWDGE (GPSIMD dma_gather) and
HWDGE (regular dma_start) descriptor generation. So:

  - Tables emb0/emb1 use per-token dma_gather (2048-idx per chunk, the
    per-descriptor sweet spot); emb2 (200x16) / emb3 (50x8) are gathered
    with TensorE one-hot matmuls (bf16, exact row-select, ~0.4% quant err
    vs the 2e-2 gate), removing half the SWDGE descriptors.
  - Every dma_start is made contiguous-per-partition so HWDGE descriptor
    counts collapse (~185k -> ~15k): the host marshals pre-wrapped index
    arrays (idsw0/idsw1), a j-ordered id row pair (ids23), a p-major
    numeric block (dsnum), and pre-wrapped luts; tokens are assigned to
    gather positions p-major (token = p*256 + c*16 + slot) so each output
    store is one 8KB contiguous run per partition (128 descs vs 2048).
"""

import numpy as np

B, T = 64, 4096
NCORES = 8
BC = B // NCORES            # batch rows per core
N = BC * T                  # 32768 tokens per core
NCOLS = 12
VOCABS = [1000, 5000, 200, 50]
DIMS = [32, 64, 16, 8]
OFF = [0, 32, 96, 112]
NUM_OFF = 120
DOUT = 128
PAD = 64                    # padded row length (f32) = 256B
PROWS = 8192
VPAD = [((v + 127) // 128) * 128 for v in VOCABS]   # 1024, 5120, 256, 128
LUT_SLOT_OFF = [0, 64, 384, 400]
LUT_REGION = 512

NCHUNK = 16
CH = N // NCHUNK            # 2048 tokens per chunk
IPP = CH // 128             # 16 out slots per partition per chunk
SPC = CH // 16              # 128 idx slots per table per chunk
TOK_SLOTS = 2 * (N // 16)
W16 = TOK_SLOTS + LUT_REGION

_CACHE = {}

SCRATCH = 65536
GBUFS = 4
OBUFS = 2
NQUEUES = 1


def _build_program(reps=1):
    from contextlib import ExitStack

    import concourse.bacc as bacc
    import concourse.tile as tile
    from concourse import mybir
    from concourse.tile import add_dep_helper

    F32, I32, I16 = mybir.dt.float32, mybir.dt.int32, mybir.dt.int16
    BF16, F16 = mybir.dt.bfloat16, mybir.dt.float16

    nc = bacc.Bacc("TRN2", target_bir_lowering=False, debug=False,
                   num_devices=NCORES, dynamic_dma_scratch_size=SCRATCH,
                   num_swdge_queues=NQUEUES)
    idsw_d = [nc.dram_tensor(f"idsw{k}", [16, N // 16], I32, kind="ExternalInput")
              for k in range(2)]
    ids23_d = nc.dram_tensor("ids23", [2, N], F16, kind="ExternalInput")
    dsnum_d = nc.dram_tensor("dsnum", [128, N // 128, 8], F32,
                             kind="ExternalInput")
    lutw_d = [nc.dram_tensor(f"lutw{k}", [16, VPAD[k] // 16], I32,
                             kind="ExternalInput")
              for k in range(4)]
    out = nc.dram_tensor("out", [N, DOUT], F32, kind="ExternalOutput")
    embs = [
        nc.dram_tensor(f"emb{k}", [VOCABS[k], DIMS[k]], F32, kind="ExternalInput")
        for k in range(4)
    ]
    pembs = [nc.dram_tensor(f"pemb{k}", [PROWS, PAD], F32) for k in range(4)]
    cembs = [nc.dram_tensor(f"cemb{k}", [PROWS, PAD], F32) for k in range(2)]

    with tile.TileContext(nc) as tc:
        with ExitStack() as ctx:
            sm_pool = ctx.enter_context(tc.tile_pool(name="small", bufs=1))
            # big16 is read by every gather until the rep's end; double-buffer
            # it so the next rep's index build + compose overlap this rep's
            # token gathers instead of stalling the Pool engine.
            b16_pool = ctx.enter_context(tc.tile_pool(name="b16", bufs=2))
            comp_pool = ctx.enter_context(tc.tile_pool(name="comp", bufs=1))
            nds_pool = ctx.enter_context(tc.tile_pool(name="nds", bufs=1))
            g_pool = ctx.enter_context(tc.tile_pool(name="gt", bufs=GBUFS))
            o_pool = ctx.enter_context(tc.tile_pool(name="ot", bufs=OBUFS))
            idr_pool = ctx.enter_context(tc.tile_pool(name="idr", bufs=2))
            rep_pool = ctx.enter_context(tc.tile_pool(name="idsrep", bufs=2))
            oh_pool = ctx.enter_context(tc.tile_pool(name="oh", bufs=2))
            ps_pool = ctx.enter_context(
                tc.tile_pool(name="ps", bufs=2, space="PSUM"))

            def one_pass():
                # ---------- Stage A ----------
                pemb_cp = []
                for k in range(4):
                    w = nc.sync.dma_start(
                        out=pembs[k].ap()[: VOCABS[k], : DIMS[k]],
                        in_=embs[k].ap(),
                    )
                    pemb_cp.append(w)

                big16 = b16_pool.tile([128, W16], I16, name="big16")
                nc.vector.memset(big16[:16, TOK_SLOTS + 408 :], 0)

                # luts: host-wrapped int32 -> int16
                for k in range(4):
                    nslot = VPAD[k] // 16
                    wlut32 = sm_pool.tile([16, nslot], I32, name=f"wlut32_{k}")
                    nc.sync.dma_start(out=wlut32[:], in_=lutw_d[k].ap())
                    lo = TOK_SLOTS + LUT_SLOT_OFF[k]
                    nc.vector.tensor_copy(
                        out=big16[:16, lo : lo + nslot], in_=wlut32[:]
                    )

                # token ids for t0/t1: host-wrapped int32 -> int16
                for k in range(2):
                    widx32 = sm_pool.tile([16, N // 16], I32, name=f"widx32_{k}")
                    nc.sync.dma_start(out=widx32[:], in_=idsw_d[k].ap())
                    nc.vector.tensor_copy(
                        out=big16[:16, k * (N // 16) : (k + 1) * (N // 16)],
                        in_=widx32[:],
                    )

                nc.sync.dma_start(out=big16[16:32, :], in_=big16[0:16, :])
                nc.sync.dma_start(out=big16[32:64, :], in_=big16[0:32, :])
                nc.sync.dma_start(out=big16[64:128, :], in_=big16[0:64, :])

                # compose cemb_k = pemb_k[lut_k]
                comp_ts = []
                wb = []
                for k in range(4):
                    nslot = VPAD[k] // 16
                    lo = TOK_SLOTS + LUT_SLOT_OFF[k]
                    comp_t = comp_pool.tile(
                        [128, VPAD[k] // 128, PAD], F32, name=f"comp_t{k}"
                    )
                    cg = nc.gpsimd.dma_gather(
                        comp_t[:],
                        pembs[k].ap(),
                        big16[:, lo : lo + nslot],
                        VPAD[k],
                        VPAD[k],
                        PAD,
                        single_packet=False,
                    )
                    add_dep_helper(cg.ins, pemb_cp[k].ins, reason=f"pemb{k} RAW")
                    comp_ts.append(comp_t)
                    if k < 2:
                        w = nc.sync.dma_start(
                            out=cembs[k].ap()[: VPAD[k], :].rearrange(
                                "(i p) d -> p i d", p=128
                            ),
                            in_=comp_t[:],
                        )
                        wb.append(w)

                # bf16 moving operands for the PE path
                mv2a = sm_pool.tile([128, 16], BF16, name="mv2a")
                nc.vector.tensor_copy(out=mv2a[:], in_=comp_ts[2][:, 0, :16])
                mv2b = sm_pool.tile([128, 16], BF16, name="mv2b")
                nc.vector.tensor_copy(out=mv2b[:72, :], in_=comp_ts[2][0:72, 1, :16])
                mv3 = sm_pool.tile([128, 8], BF16, name="mv3")
                nc.vector.tensor_copy(out=mv3[:50, :], in_=comp_ts[3][0:50, 0, :8])

                io0 = sm_pool.tile([128, 1], F32, name="io0")
                nc.gpsimd.iota(io0[:], pattern=[[0, 1]], base=0,
                               channel_multiplier=1,
                               allow_small_or_imprecise_dtypes=True)
                io1 = sm_pool.tile([128, 1], F32, name="io1")
                nc.gpsimd.iota(io1[:], pattern=[[0, 1]], base=128,
                               channel_multiplier=1,
                               allow_small_or_imprecise_dtypes=True)

                # numeric features (host p-major block, contiguous load)
                nds = nds_pool.tile([128, N // 128, 8], F32, name="nds")
                nc.sync.dma_start(out=nds[:], in_=dsnum_d.ap())

                out_pm = out.ap().rearrange("(p i) f -> p i f", p=128)

                # ---------- Stage B ----------
                for c in range(NCHUNK):
                    o_t = o_pool.tile([128, IPP, DOUT], F32, name="o_t")
                    nc.scalar.copy(
                        out=o_t[:, :, NUM_OFF:],
                        in_=nds[:, c * IPP : (c + 1) * IPP, :],
                    )
                    for k in range(2):
                        g_t = g_pool.tile([128, IPP, PAD], F32, name="g_t")
                        so = k * (N // 16) + c * SPC
                        gi = nc.gpsimd.dma_gather(
                            g_t[:],
                            cembs[k].ap(),
                            big16[:, so : so + SPC],
                            CH,
                            CH,
                            PAD,
                            single_packet=False,
                        )
                        add_dep_helper(gi.ins, wb[k].ins, reason=f"cemb{k} RAW")
                        if k == 0:
                            nc.vector.tensor_copy(
                                out=o_t[:, :, OFF[0] : OFF[0] + DIMS[0]],
                                in_=g_t[:, :, : DIMS[0]],
                            )
                        else:
                            nc.scalar.copy(
                                out=o_t[:, :, OFF[1] : OFF[1] + DIMS[1]],
                                in_=g_t[:, :, : DIMS[1]],
                            )

                    # small tables via one-hot matmul
                    idr = idr_pool.tile([2, CH], F16, name="idr")
                    nc.sync.dma_start(
                        out=idr[:], in_=ids23_d.ap()[:, c * CH : (c + 1) * CH]
                    )
                    rep2 = rep_pool.tile([128, CH], F16, name="rep2")
                    rep3 = rep_pool.tile([128, CH], F16, name="rep3")
                    nc.sync.dma_start(out=rep2[0:1, :], in_=idr[0:1, :])
                    nc.sync.dma_start(out=rep3[0:1, :], in_=idr[1:2, :])
                    for m in (1, 2, 4, 8, 16, 32, 64):
                        nc.sync.dma_start(out=rep2[m : 2 * m, :], in_=rep2[0:m, :])
                        nc.scalar.dma_start(out=rep3[m : 2 * m, :], in_=rep3[0:m, :])

                    oh2a = oh_pool.tile([128, CH], BF16, name="oh2a")
                    nc.vector.tensor_scalar(
                        out=oh2a[:], in0=rep2[:], scalar1=io0[:], scalar2=None,
                        op0=mybir.AluOpType.is_equal,
                    )
                    oh2b = oh_pool.tile([128, CH], BF16, name="oh2b")
                    nc.vector.tensor_scalar(
                        out=oh2b[0:72, :], in0=rep2[0:72, :], scalar1=io1[0:72, :],
                        scalar2=None, op0=mybir.AluOpType.is_equal,
                    )
                    oh3 = oh_pool.tile([128, CH], BF16, name="oh3")
                    nc.vector.tensor_scalar(
                        out=oh3[0:50, :], in0=rep3[0:50, :], scalar1=io0[0:50, :],
                        scalar2=None, op0=mybir.AluOpType.is_equal,
                    )

                    pp = ps_pool.tile([128, IPP, 24], F32, name="pp")
                    for ti in range(IPP):
                        ts = ti * 128
                        nc.tensor.matmul(
                            pp[:, ti, 0:16],
                            oh2a[:, ts : ts + 128],
                            mv2a[:],
                            start=True, stop=False, skip_group_check=True,
                        )
                        nc.tensor.matmul(
                            pp[:, ti, 0:16],
                            oh2b[0:72, ts : ts + 128],
                            mv2b[0:72, :],
                            start=False, stop=True, skip_group_check=True,
                        )
                        nc.tensor.matmul(
                            pp[:, ti, 16:24],
                            oh3[0:50, ts : ts + 128],
                            mv3[0:50, :],
                            start=True, stop=True, skip_group_check=True,
                        )
                    nc.scalar.copy(
                        out=o_t[:, :, OFF[2] : OFF[2] + 24], in_=pp[:]
                    )

                    nc.sync.dma_start(
                        out=out_pm[:, c * IPP : (c + 1) * IPP, :],
                        in_=o_t[:],
                    )

            for _rep in range(reps):
                one_pass()
    nc.compile()
    return nc


def get_program():
    if "nc" not in _CACHE:
        _CACHE["nc"] = _build_program()
    return _CACHE["nc"]


# token assigned to gather position: chunk c, in-chunk position j ->
# token (j%128)*256 + c*16 + j//128; global position g = c*2048 + j.
def _token_perm():
    g = np.arange(N)
    c, j = g // CH, g % CH
    return (j % 128) * 256 + c * IPP + j // 128


_TPERM = _token_perm()


def make_in_maps(inputs):
    dataset = np.asarray(inputs["dataset"], dtype=np.float32)
    in_maps = []
    for i in range(NCORES):
        dsc = dataset[i * BC : (i + 1) * BC].reshape(N, NCOLS)
        m = {}
        ids01 = dsc[:, 0:2].astype(np.int32)
        for k in range(2):
            x = ids01[_TPERM, k]
            m[f"idsw{k}"] = np.ascontiguousarray(x.reshape(N // 16, 16).T)
        m["ids23"] = np.ascontiguousarray(dsc[_TPERM, 2:4].T.astype(np.float16))
        m["dsnum"] = np.ascontiguousarray(dsc[:, 4:NCOLS].reshape(128, N // 128, 8))
        for k in range(4):
            lut = np.zeros(VPAD[k], dtype=np.int32)
            lut[: VOCABS[k]] = np.asarray(inputs[f"lut{k}"], dtype=np.int32)
            m[f"lutw{k}"] = np.ascontiguousarray(lut.reshape(VPAD[k] // 16, 16).T)
            m[f"emb{k}"] = np.ascontiguousarray(inputs[f"emb{k}"], dtype=np.float32)
        in_maps.append(m)
    return in_maps


def kernel(**inputs):
    from concourse.bass_utils import run_bass_kernel_spmd

    nc = get_program()
    in_maps = make_in_maps(inputs)
    res = run_bass_kernel_spmd(nc, in_maps, list(range(NCORES))).results
    outs = [np.asarray(res[i]["out"]).reshape(BC, T, DOUT) for i in range(NCORES)]
    return np.concatenate(outs, axis=0)
